# revision 24
# baseline (speedup 1.0000x reference)
"""Trainium2 Bass kernel for KANPolyLayer:
    y[b,o] = sum_{i,p} x[b,i]^p * coeffs[o,i,p] + bias[o],  p = 0..4

Math: y = sum_{p=1..4} (x^p) @ C_p^T + biascol, with C_p = coeffs[:,:,p]
and biascol[o] = bias[o] + sum_i coeffs[o,i,0] folded on host (the p=0
plane is a constant column; folding it is 0.003% of the FLOPs).

Two-phase mixed-precision stream (the p1/p2 planes carry only ~3% of
the output variance, so fp8 there costs little accuracy but halves
their PE time via DoubleRow):

  Phase A (fp8 DoubleRow): y12 = p1 @ C1'^T + p2 @ C2'^T with
    C' = 256*C cast to fp8e4m3 on host; on-chip p1 = DVE cast of the
    bf16 x plane, p2 = ACT square straight to fp8.  DoubleRow packs
    the (p1,p2) pair as one 256-deep contraction per matmul at 2 fp8
    cols/cycle -> 64 matmuls instead of 128.  Per-bank mid-evacuation
    folds the 1/256 descale and the bias into one ACT:
    stash = psA * (1/256) + biascol.
  Phase B (bf16): psB accumulates p3/p4 (DVE muls from a second,
    single-descriptor x copy); final DVE add of psB + stash.

Measured end-to-end rel err ~1.0e-2 (gate 2e-2; verified against an
offline simulation of the same rounding chain).

Schedule notes (from perfetto/NTFF analysis):
- The DMA engines fair-share across all in-flight descriptors, so the
  first tiles' arrival is ruled by how few descriptors are in flight:
  each queue opens with a tiny critical descriptor and later bulk
  loads are issued behind data-gated compute (scalar squares wait on
  x, so descriptors queued after them cannot flood the early window).
- A-side x k-planes ride a bufs=3 ring on the SP queue; only the fp8
  casts/squares consume them, so the ring self-throttles at phase-A
  pace.  Phase B reads its own whole-x copy (one wide descriptor).
- Engine balance: phase A: scalar = squares-to-fp8 (1.44us/k),
  vector = p1 casts (0.86us/k), PE eats 1.73us/k.  Phase B: vector =
  p2/p3/p4 muls (2.5us/k) vs PE 3.46us/k.  Mid-evacs on scalar.
- PE warmup matmuls read broadcast const-APs (written by the framework
  preamble) so they start the moment the PE exits the preamble and
  the HAM clock-gate reaches 2.4 GHz as the real stream begins.
- 8 PSUM banks (4 o-tiles x 2 b-halves); each phase's last NTAIL
  k-planes are emitted bank-contiguous so banks finish staggered and
  evacuation overlaps the stream.

The kernel computes yT = [o, b]; host transposes.

Sharding (8 cores): 4 batch groups x 2 out-dim groups.
  core c -> (bg, og) = (c // 2, c % 2)
Each core computes a disjoint (512 x 1024) block of yT; host gathers.
"""

from contextlib import ExitStack

import ml_dtypes
import numpy as np

import concourse.bacc as bacc
import concourse.bass as bass
import concourse.mybir as mybir
import concourse.tile as tile
from concourse.bass_utils import run_bass_kernel_spmd

F32 = mybir.dt.float32
BF16 = mybir.dt.bfloat16
FP8 = mybir.dt.float8e4

B, I, O = 4096, 1024, 1024  # batch, in_dim, out_dim
BW, OW = 4, 2               # batch groups x out-dim groups (8 cores)
BS, OS = B // BW, O // OW   # per-core batch (1024) and out (512)
NK = I // 128               # contraction tiles (8)
NT = OS // 128              # o-tiles (4)
NH = BS // 512              # b-halves (2)
NTAIL = 2                   # trailing k-planes emitted bank-contiguous
WN = 13                     # PE warmup matmuls (HAM clock-gate)
CSC = 256.0                 # fp8 coeff scale (power of 2, exact)

_CACHE: dict = {}


def _build():
    nc = bacc.Bacc("TRN2", target_bir_lowering=False, debug=False, num_devices=8)

    # x partition-major [i, k*b]: one resident tile serves both phases
    xt2 = nc.dram_tensor("xt2", [128, NK * BS], BF16, kind="ExternalInput")
    # fp8 coeffs for phase A: [i, k, ot, p12, o] = 256*C{1,2}
    c8 = nc.dram_tensor("c8", [128, NK, NT, 2, 128], FP8, kind="ExternalInput")
    # bf16 coeffs for phase B: [i, k*p34*o] partition-major
    cb = nc.dram_tensor("cb", [128, NK * 2 * OS], BF16, kind="ExternalInput")
    # [i, ot]: bias[o] + colsum(C0)[o] as per-partition scalars
    biasc = nc.dram_tensor("biasc", [128, NT], F32, kind="ExternalInput")
    yt = nc.dram_tensor("yt", [OS, BS], F32, kind="ExternalOutput")  # [o, b]

    with tile.TileContext(nc) as tc, ExitStack() as ctx:
        cons = ctx.enter_context(tc.tile_pool(name="cons", bufs=1))
        c8pool = ctx.enter_context(tc.tile_pool(name="c8p", bufs=1))
        cbpool = ctx.enter_context(tc.tile_pool(name="cbp", bufs=1))
        x2pool = ctx.enter_context(tc.tile_pool(name="x2in", bufs=1))
        ppool = ctx.enter_context(tc.tile_pool(name="pow", bufs=1))
        spool = ctx.enter_context(tc.tile_pool(name="stash", bufs=1))
        opool = ctx.enter_context(tc.tile_pool(name="out", bufs=3))
        pspool = ctx.enter_context(
            tc.tile_pool(name="ps", bufs=8, space=bass.MemorySpace.PSUM)
        )

        # 8 accumulation banks: (o-tile, b-half)
        ps = {}
        for ot in range(NT):
            for h in range(NH):
                ps[(ot, h)] = pspool.tile(
                    [128, 512], F32, tag="ps", name=f"ps_{ot}_{h}"
                )

        # PE warmup on framework-preamble const tiles (no memset dep)
        wl = nc.const_aps.tensor(1.0, [128, 128], BF16)
        wr = nc.const_aps.tensor(1.0, [128, 512], BF16)
        for w in range(WN):
            nc.tensor.matmul(
                ps[(0, 0)], wl, wr, start=True, stop=True,
                skip_group_check=True,
            )

        biasc_sb = cons.tile([128, NT], F32)

        # --- DMA: tiny critical descriptors first, bulk gated behind
        # data-dependent compute (scalar squares wait on x, so descriptors
        # queued after them cannot flood the early window) ---
        c8t = c8pool.tile([128, NK, NT, 2, 128], FP8, tag="c8t", name="c8t")
        cbt = cbpool.tile([128, NK * 2 * OS], BF16, tag="cbt", name="cbt")
        x2t = x2pool.tile([128, NK * BS], BF16, tag="x2t", name="x2t")

        # both queues open with an x plane (first data = most critical);
        # fp8 coeffs staged fine -> coarse behind them on the ACT queue
        nc.sync.dma_start(x2t[:, 0:BS], xt2[:, 0:BS])
        nc.scalar.dma_start(x2t[:, BS:2 * BS], xt2[:, BS:2 * BS])
        nc.scalar.dma_start(c8t[:, 0, 0], c8[:, 0, 0])          # k0/ot0, 32KB
        nc.scalar.dma_start(c8t[:, 0, 1:NT], c8[:, 0, 1:NT])    # k0 rest
        nc.scalar.dma_start(c8t[:, 1], c8[:, 1])                # k1
        nc.scalar.dma_start(c8t[:, 2:4], c8[:, 2:4])            # k2-3

        # ---------------- phase A: fp8 DoubleRow (p1, p2) ----------------
        pw12 = {}   # (k, h) -> [128, 2, 512] fp8
        p2bs = {}   # (k, h) -> [128, 512] bf16
        for k in range(NK):
            for h in range(NH):
                sl = x2t[:, k * BS + h * 512:k * BS + (h + 1) * 512]
                pw = ppool.tile([128, 2, 512], FP8, tag=f"pw_{k}_{h}",
                                name=f"pw_{k}_{h}")
                nc.vector.tensor_copy(pw[:, 0], sl)   # fp8 cast of x
                nc.scalar.square(pw[:, 1], sl)        # x^2 -> fp8 direct
                p2b = ppool.tile([128, 512], BF16, tag=f"p2_{k}_{h}",
                                 name=f"p2_{k}_{h}")
                nc.vector.tensor_mul(p2b[:], sl, sl)  # x^2 bf16 for phase B
                pw12[(k, h)] = pw
                p2bs[(k, h)] = p2b

            # staged loads behind the k-gated squares (in-order ACT queue)
            # so they never starve the phase-A-critical stream; phase-B
            # coeffs mostly load during phase B (its DMA window is idle)
            if k == 0:
                nc.scalar.dma_start(x2t[:, 2 * BS:3 * BS],
                                    xt2[:, 2 * BS:3 * BS])
            if k == 1:
                nc.scalar.dma_start(x2t[:, 3 * BS:4 * BS],
                                    xt2[:, 3 * BS:4 * BS])
            if k == 2:
                nc.scalar.dma_start(c8t[:, 4:NK], c8[:, 4:NK])      # 512KB
                nc.scalar.dma_start(x2t[:, 4 * BS:6 * BS],
                                    xt2[:, 4 * BS:6 * BS])
            if k == 3:
                nc.scalar.dma_start(x2t[:, 6 * BS:NK * BS],
                                    xt2[:, 6 * BS:NK * BS])
            if k == 4:
                nc.scalar.dma_start(biasc_sb[:], biasc[:])
            if k == 5:
                nc.scalar.dma_start(cbt[:, 0:2 * 2 * OS], cb[:, 0:2 * 2 * OS])

            if k < NK - NTAIL:
                for ot in range(NT):
                    for h in range(NH):
                        nc.tensor.matmul(
                            ps[(ot, h)],
                            c8t[:, k, ot],
                            pw12[(k, h)][:],
                            start=(k == 0),
                            stop=False,
                            perf_mode=mybir.MatmulPerfMode.DoubleRow,
                        )

        # phase-A tail, bank-contiguous + mid-evac (descale + bias, one ACT)
        stash = {}
        for ot in range(NT):
            for h in range(NH):
                for k in range(NK - NTAIL, NK):
                    nc.tensor.matmul(
                        ps[(ot, h)],
                        c8t[:, k, ot],
                        pw12[(k, h)][:],
                        start=False,
                        stop=(k == NK - 1),
                        perf_mode=mybir.MatmulPerfMode.DoubleRow,
                    )
                # mid-evac split across both PSUM-capable engines
                st = spool.tile([128, 512], F32, tag=f"st_{ot}_{h}",
                                name=f"st_{ot}_{h}")
                nc.scalar.activation(
                    st[:, 0:256], ps[(ot, h)][:, 0:256],
                    mybir.ActivationFunctionType.Identity,
                    bias=biasc_sb[:, ot:ot + 1],
                    scale=1.0 / CSC,
                )
                nc.vector.tensor_scalar(
                    st[:, 256:512], ps[(ot, h)][:, 256:512],
                    1.0 / CSC, biasc_sb[:, ot:ot + 1],
                    mybir.AluOpType.mult, mybir.AluOpType.add,
                )
                stash[(ot, h)] = st

        # rest of the phase-B coeffs: B's DMA window is otherwise idle
        nc.scalar.dma_start(cbt[:, 2 * 2 * OS:5 * 2 * OS],
                            cb[:, 2 * 2 * OS:5 * 2 * OS])
        nc.scalar.dma_start(cbt[:, 5 * 2 * OS:NK * 2 * OS],
                            cb[:, 5 * 2 * OS:NK * 2 * OS])

        # ---------------- phase B: bf16 (p3, p4) ----------------
        pows = {}
        for k in range(NK):
            for h in range(NH):
                sl2 = x2t[:, k * BS + h * 512:k * BS + (h + 1) * 512]
                p2b = p2bs[(k, h)]
                p3 = ppool.tile([128, 512], BF16, tag=f"p3_{k}_{h}",
                                name=f"p3_{k}_{h}")
                p4 = ppool.tile([128, 512], BF16, tag=f"p4_{k}_{h}",
                                name=f"p4_{k}_{h}")
                nc.vector.tensor_mul(p3[:], p2b[:], sl2)
                nc.vector.tensor_mul(p4[:], p2b[:], p2b[:])
                pows[(3, k, h)] = p3
                pows[(4, k, h)] = p4

            if k < NK - NTAIL:
                for p in (3, 4):
                    for ot in range(NT):
                        for h in range(NH):
                            nc.tensor.matmul(
                                ps[(ot, h)],
                                cbt[:, k * 2 * OS + (p - 3) * OS + ot * 128:
                                    k * 2 * OS + (p - 3) * OS + (ot + 1) * 128],
                                pows[(p, k, h)][:],
                                start=(k == 0 and p == 3),
                                stop=False,
                            )

        # phase-B tail, bank-contiguous + final add + output DMA
        ngroups = NT * NH
        gi = 0
        for ot in range(NT):
            for h in range(NH):
                for k in range(NK - NTAIL, NK):
                    for p in (3, 4):
                        nc.tensor.matmul(
                            ps[(ot, h)],
                            cbt[:, k * 2 * OS + (p - 3) * OS + ot * 128:
                                k * 2 * OS + (p - 3) * OS + (ot + 1) * 128],
                            pows[(p, k, h)][:],
                            start=False,
                            stop=(k == NK - 1 and p == 4),
                        )
                # final add in halves so each half's DMA overlaps the other
                o_sb = opool.tile([128, 512], F32, tag="o_sb", name=f"o_{ot}_{h}")
                nc.vector.tensor_add(
                    o_sb[:, 0:256], ps[(ot, h)][:, 0:256],
                    stash[(ot, h)][:, 0:256]
                )
                nc.sync.dma_start(
                    yt[ot * 128:(ot + 1) * 128, h * 512:h * 512 + 256],
                    o_sb[:, 0:256],
                )
                nc.vector.tensor_add(
                    o_sb[:, 256:512], ps[(ot, h)][:, 256:512],
                    stash[(ot, h)][:, 256:512]
                )
                nc.scalar.dma_start(
                    yt[ot * 128:(ot + 1) * 128, h * 512 + 256:(h + 1) * 512],
                    o_sb[:, 256:512],
                )
                gi += 1

    nc.compile()
    return nc


def _get_nc():
    if "nc" not in _CACHE:
        _CACHE["nc"] = _build()
    return _CACHE["nc"]


def _make_in_maps(x, coeffs, bias):
    x = np.asarray(x, dtype=np.float32)
    coeffs = np.asarray(coeffs, dtype=np.float32)
    bias = np.asarray(bias, dtype=np.float32)
    f8 = ml_dtypes.float8_e4m3

    # x slices: [1024b, 1024i] -> [1024i, 1024b] bf16, partition-major
    xt2s = [
        np.ascontiguousarray(
            x[bg * BS:(bg + 1) * BS, :].T
            .astype(ml_dtypes.bfloat16)
            .reshape(NK, 128, BS)
            .transpose(1, 0, 2)
            .reshape(128, NK * BS)
        )
        for bg in range(BW)
    ]
    c8s, cbs, biascs = [], [], []
    for og in range(OW):
        C = coeffs[og * OS:(og + 1) * OS, :, :]  # [512o, 1024i, 5]
        # fp8 planes p1,p2 scaled: [1024i, 2p, 512o] -> [128,8k,4ot,2p,128o]
        c12 = np.ascontiguousarray(
            (C[:, :, 1:3] * CSC).transpose(1, 2, 0)
        ).astype(f8)  # [1024i, 2, 512o]
        c12 = c12.reshape(NK, 128, 2, NT, 128).transpose(1, 0, 3, 2, 4)
        c8s.append(np.ascontiguousarray(c12))
        # bf16 planes p3,p4: [1024i, 2p, 512o] -> [128, 8k*2p*512o]
        c34 = (
            C[:, :, 3:5].transpose(1, 2, 0)
            .astype(ml_dtypes.bfloat16)
            .reshape(NK, 128, 2 * OS)
            .transpose(1, 0, 2)
            .reshape(128, NK * 2 * OS)
        )
        cbs.append(np.ascontiguousarray(c34))
        bc = (
            bias[0, og * OS:(og + 1) * OS] + C[:, :, 0].sum(axis=1)
        ).astype(np.float32)
        biascs.append(np.ascontiguousarray(bc.reshape(NT, 128).T))
    in_maps = []
    for c in range(BW * OW):
        bg, og = c // OW, c % OW
        in_maps.append(
            {"xt2": xt2s[bg], "c8": c8s[og], "cb": cbs[og],
             "biasc": biascs[og]}
        )
    return in_maps


def _gather(results):
    y = np.empty((B, O), dtype=np.float32)
    for c, res in enumerate(results):
        bg, og = c // OW, c % OW
        y[bg * BS:(bg + 1) * BS, og * OS:(og + 1) * OS] = res["yt"].T
    return y


def run(x, coeffs, bias, trace=False, **trace_kwargs):
    nc = _get_nc()
    in_maps = _make_in_maps(x, coeffs, bias)
    br = run_bass_kernel_spmd(
        nc, in_maps, list(range(BW * OW)), trace=trace, **trace_kwargs
    )
    return _gather(br.results), br


def kernel(x, coeffs, bias):
    out, _ = run(x, coeffs, bias)
    return out


# revision 25
# speedup vs baseline: 1.0509x; 1.0509x over previous
"""Trainium2 Bass kernel for KANPolyLayer:
    y[b,o] = sum_{i,p} x[b,i]^p * coeffs[o,i,p] + bias[o],  p = 0..4

Math: y = sum_{p=1..4} (x^p) @ C_p^T + biascol, with C_p = coeffs[:,:,p]
and biascol[o] = bias[o] + sum_i coeffs[o,i,0] folded on host (the p=0
plane is a constant column; folding it is 0.003% of the FLOPs).

Two-phase mixed-precision stream (the p1/p2 planes carry only ~3% of
the output variance, so fp8 there costs little accuracy but halves
their PE time via DoubleRow):

  Phase A (fp8 DoubleRow): y12 = p1 @ C1'^T + p2 @ C2'^T with
    C' = 256*C cast to fp8e4m3 on host; on-chip p1 = DVE cast of the
    bf16 x plane, p2 = ACT square straight to fp8.  DoubleRow packs
    the (p1,p2) pair as one 256-deep contraction per matmul at 2 fp8
    cols/cycle -> 64 matmuls instead of 128.  Per-bank mid-evacuation
    folds the 1/256 descale and the bias into one ACT:
    stash = psA * (1/256) + biascol.
  Phase B (bf16): psB accumulates p3/p4 (DVE muls from a second,
    single-descriptor x copy); final DVE add of psB + stash.

Measured end-to-end rel err ~1.0e-2 (gate 2e-2; verified against an
offline simulation of the same rounding chain).

Schedule notes (from perfetto/NTFF analysis):
- The DMA engines fair-share across all in-flight descriptors, so the
  first tiles' arrival is ruled by how few descriptors are in flight:
  each queue opens with a tiny critical descriptor and later bulk
  loads are issued behind data-gated compute (scalar squares wait on
  x, so descriptors queued after them cannot flood the early window).
- A-side x k-planes ride a bufs=3 ring on the SP queue; only the fp8
  casts/squares consume them, so the ring self-throttles at phase-A
  pace.  Phase B reads its own whole-x copy (one wide descriptor).
- Engine balance: phase A: scalar = squares-to-fp8 (1.44us/k),
  vector = p1 casts (0.86us/k), PE eats 1.73us/k.  Phase B: vector =
  p2/p3/p4 muls (2.5us/k) vs PE 3.46us/k.  Mid-evacs on scalar.
- PE warmup matmuls read broadcast const-APs (written by the framework
  preamble) so they start the moment the PE exits the preamble and
  the HAM clock-gate reaches 2.4 GHz as the real stream begins.
- 8 PSUM banks (4 o-tiles x 2 b-halves); each phase's last NTAIL
  k-planes are emitted bank-contiguous so banks finish staggered and
  evacuation overlaps the stream.

The kernel computes yT = [o, b]; host transposes.

Sharding (8 cores): 4 batch groups x 2 out-dim groups.
  core c -> (bg, og) = (c // 2, c % 2)
Each core computes a disjoint (512 x 1024) block of yT; host gathers.
"""

from contextlib import ExitStack

import ml_dtypes
import numpy as np

import concourse.bacc as bacc
import concourse.bass as bass
import concourse.mybir as mybir
import concourse.tile as tile
from concourse.bass_utils import run_bass_kernel_spmd

F32 = mybir.dt.float32
BF16 = mybir.dt.bfloat16
FP8 = mybir.dt.float8e4

B, I, O = 4096, 1024, 1024  # batch, in_dim, out_dim
BW, OW = 4, 2               # batch groups x out-dim groups (8 cores)
BS, OS = B // BW, O // OW   # per-core batch (1024) and out (512)
NK = I // 128               # contraction tiles (8)
NT = OS // 128              # o-tiles (4)
NH = BS // 512              # b-halves (2)
NTAIL = 2                   # trailing k-planes emitted bank-contiguous
WN = 13                     # PE warmup matmuls (HAM clock-gate)
CSC = 256.0                 # fp8 coeff scale (power of 2, exact)

_CACHE: dict = {}


def _build():
    nc = bacc.Bacc("TRN2", target_bir_lowering=False, debug=False, num_devices=8)

    # x partition-major [i, k*b]: one resident tile serves both phases
    xt2 = nc.dram_tensor("xt2", [128, NK * BS], BF16, kind="ExternalInput")
    # fp8 coeffs for phase A: [i, k, ot, p12, o] = 256*C{1,2}
    c8 = nc.dram_tensor("c8", [128, NK, NT, 2, 128], FP8, kind="ExternalInput")
    # bf16 coeffs for phase B: [i, k*p34*o] partition-major
    cb = nc.dram_tensor("cb", [128, NK * 2 * OS], BF16, kind="ExternalInput")
    # [i, ot]: bias[o] + colsum(C0)[o] as per-partition scalars
    biasc = nc.dram_tensor("biasc", [128, NT], F32, kind="ExternalInput")
    yt = nc.dram_tensor("yt", [OS, BS], F32, kind="ExternalOutput")  # [o, b]

    with tile.TileContext(nc) as tc, ExitStack() as ctx:
        cons = ctx.enter_context(tc.tile_pool(name="cons", bufs=1))
        c8pool = ctx.enter_context(tc.tile_pool(name="c8p", bufs=1))
        cbpool = ctx.enter_context(tc.tile_pool(name="cbp", bufs=1))
        x2pool = ctx.enter_context(tc.tile_pool(name="x2in", bufs=1))
        ppool = ctx.enter_context(tc.tile_pool(name="pow", bufs=1))
        spool = ctx.enter_context(tc.tile_pool(name="stash", bufs=1))
        opool = ctx.enter_context(tc.tile_pool(name="out", bufs=3))
        pspool = ctx.enter_context(
            tc.tile_pool(name="ps", bufs=8, space=bass.MemorySpace.PSUM)
        )

        # 8 accumulation banks: (o-tile, b-half)
        ps = {}
        for ot in range(NT):
            for h in range(NH):
                ps[(ot, h)] = pspool.tile(
                    [128, 512], F32, tag="ps", name=f"ps_{ot}_{h}"
                )

        # PE warmup on framework-preamble const tiles (no memset dep)
        wl = nc.const_aps.tensor(1.0, [128, 128], BF16)
        wr = nc.const_aps.tensor(1.0, [128, 512], BF16)
        for w in range(WN):
            nc.tensor.matmul(
                ps[(0, 0)], wl, wr, start=True, stop=True,
                skip_group_check=True,
            )

        biasc_sb = cons.tile([128, NT], F32)

        # --- DMA: tiny critical descriptors first, bulk gated behind
        # data-dependent compute (scalar squares wait on x, so descriptors
        # queued after them cannot flood the early window) ---
        c8t = c8pool.tile([128, NK, NT, 2, 128], FP8, tag="c8t", name="c8t")
        cbt = cbpool.tile([128, NK * 2 * OS], BF16, tag="cbt", name="cbt")
        x2t = x2pool.tile([128, NK * BS], BF16, tag="x2t", name="x2t")

        # both queues open with an x plane (first data = most critical);
        # fp8 coeffs staged fine -> coarse behind them on the ACT queue
        nc.sync.dma_start(x2t[:, 0:BS], xt2[:, 0:BS])
        nc.scalar.dma_start(x2t[:, BS:2 * BS], xt2[:, BS:2 * BS])
        nc.scalar.dma_start(c8t[:, 0, 0], c8[:, 0, 0])          # k0/ot0, 32KB
        nc.scalar.dma_start(c8t[:, 0, 1:NT], c8[:, 0, 1:NT])    # k0 rest
        nc.scalar.dma_start(c8t[:, 1], c8[:, 1])                # k1
        nc.scalar.dma_start(c8t[:, 2:4], c8[:, 2:4])            # k2-3

        # ---------------- phase A: fp8 DoubleRow (p1, p2) ----------------
        pw12 = {}   # (k, h) -> [128, 2, 512] fp8
        p2bs = {}   # (k, h) -> [128, 512] bf16
        for k in range(NK):
            for h in range(NH):
                sl = x2t[:, k * BS + h * 512:k * BS + (h + 1) * 512]
                pw = ppool.tile([128, 2, 512], FP8, tag=f"pw_{k}_{h}",
                                name=f"pw_{k}_{h}")
                nc.vector.tensor_copy(pw[:, 0], sl)   # fp8 cast of x
                nc.scalar.square(pw[:, 1], sl)        # x^2 -> fp8 direct
                p2b = ppool.tile([128, 512], BF16, tag=f"p2_{k}_{h}",
                                 name=f"p2_{k}_{h}")
                nc.vector.tensor_mul(p2b[:], sl, sl)  # x^2 bf16 for phase B
                pw12[(k, h)] = pw
                p2bs[(k, h)] = p2b

            # staged loads behind the k-gated squares (in-order ACT queue)
            # so they never starve the phase-A-critical stream; phase-B
            # coeffs mostly load during phase B (its DMA window is idle)
            if k == 0:
                nc.scalar.dma_start(x2t[:, 2 * BS:3 * BS],
                                    xt2[:, 2 * BS:3 * BS])
            if k == 1:
                nc.scalar.dma_start(x2t[:, 3 * BS:4 * BS],
                                    xt2[:, 3 * BS:4 * BS])
            if k == 2:
                nc.scalar.dma_start(c8t[:, 4:NK], c8[:, 4:NK])      # 512KB
                nc.scalar.dma_start(x2t[:, 4 * BS:6 * BS],
                                    xt2[:, 4 * BS:6 * BS])
            if k == 3:
                nc.scalar.dma_start(x2t[:, 6 * BS:NK * BS],
                                    xt2[:, 6 * BS:NK * BS])
            if k == 4:
                nc.scalar.dma_start(biasc_sb[:], biasc[:])
            if k == 5:
                nc.scalar.dma_start(cbt[:, 0:2 * 2 * OS], cb[:, 0:2 * 2 * OS])

            if k < NK - NTAIL:
                for ot in range(NT):
                    for h in range(NH):
                        nc.tensor.matmul(
                            ps[(ot, h)],
                            c8t[:, k, ot],
                            pw12[(k, h)][:],
                            start=(k == 0),
                            stop=False,
                            perf_mode=mybir.MatmulPerfMode.DoubleRow,
                        )

        # phase-A tail, bank-contiguous + mid-evac (descale + bias, one ACT)
        stash = {}
        for ot in range(NT):
            for h in range(NH):
                for k in range(NK - NTAIL, NK):
                    nc.tensor.matmul(
                        ps[(ot, h)],
                        c8t[:, k, ot],
                        pw12[(k, h)][:],
                        start=False,
                        stop=(k == NK - 1),
                        perf_mode=mybir.MatmulPerfMode.DoubleRow,
                    )
                st = spool.tile([128, 512], F32, tag=f"st_{ot}_{h}",
                                name=f"st_{ot}_{h}")
                nc.scalar.activation(
                    st[:], ps[(ot, h)][:],
                    mybir.ActivationFunctionType.Identity,
                    bias=biasc_sb[:, ot:ot + 1],
                    scale=1.0 / CSC,
                )
                stash[(ot, h)] = st

        # rest of the phase-B coeffs: B's DMA window is otherwise idle
        nc.scalar.dma_start(cbt[:, 2 * 2 * OS:5 * 2 * OS],
                            cb[:, 2 * 2 * OS:5 * 2 * OS])
        nc.scalar.dma_start(cbt[:, 5 * 2 * OS:NK * 2 * OS],
                            cb[:, 5 * 2 * OS:NK * 2 * OS])

        # ---------------- phase B: bf16 (p3, p4) ----------------
        pows = {}
        for k in range(NK):
            for h in range(NH):
                sl2 = x2t[:, k * BS + h * 512:k * BS + (h + 1) * 512]
                p2b = p2bs[(k, h)]
                p3 = ppool.tile([128, 512], BF16, tag=f"p3_{k}_{h}",
                                name=f"p3_{k}_{h}")
                p4 = ppool.tile([128, 512], BF16, tag=f"p4_{k}_{h}",
                                name=f"p4_{k}_{h}")
                nc.vector.tensor_mul(p3[:], p2b[:], sl2)
                nc.vector.tensor_mul(p4[:], p2b[:], p2b[:])
                pows[(3, k, h)] = p3
                pows[(4, k, h)] = p4

            if k < NK - NTAIL:
                for p in (3, 4):
                    for ot in range(NT):
                        for h in range(NH):
                            nc.tensor.matmul(
                                ps[(ot, h)],
                                cbt[:, k * 2 * OS + (p - 3) * OS + ot * 128:
                                    k * 2 * OS + (p - 3) * OS + (ot + 1) * 128],
                                pows[(p, k, h)][:],
                                start=(k == 0 and p == 3),
                                stop=False,
                            )

        # phase-B tail, bank-contiguous + final add + output DMA
        ngroups = NT * NH
        gi = 0
        for ot in range(NT):
            for h in range(NH):
                for k in range(NK - NTAIL, NK):
                    for p in (3, 4):
                        nc.tensor.matmul(
                            ps[(ot, h)],
                            cbt[:, k * 2 * OS + (p - 3) * OS + ot * 128:
                                k * 2 * OS + (p - 3) * OS + (ot + 1) * 128],
                            pows[(p, k, h)][:],
                            start=False,
                            stop=(k == NK - 1 and p == 4),
                        )
                # final add in halves so each half's DMA overlaps the other
                o_sb = opool.tile([128, 512], F32, tag="o_sb", name=f"o_{ot}_{h}")
                nc.vector.tensor_add(
                    o_sb[:, 0:256], ps[(ot, h)][:, 0:256],
                    stash[(ot, h)][:, 0:256]
                )
                nc.sync.dma_start(
                    yt[ot * 128:(ot + 1) * 128, h * 512:h * 512 + 256],
                    o_sb[:, 0:256],
                )
                nc.vector.tensor_add(
                    o_sb[:, 256:512], ps[(ot, h)][:, 256:512],
                    stash[(ot, h)][:, 256:512]
                )
                nc.scalar.dma_start(
                    yt[ot * 128:(ot + 1) * 128, h * 512 + 256:(h + 1) * 512],
                    o_sb[:, 256:512],
                )
                gi += 1

    nc.compile()
    return nc


def _get_nc():
    if "nc" not in _CACHE:
        _CACHE["nc"] = _build()
    return _CACHE["nc"]


def _make_in_maps(x, coeffs, bias):
    x = np.asarray(x, dtype=np.float32)
    coeffs = np.asarray(coeffs, dtype=np.float32)
    bias = np.asarray(bias, dtype=np.float32)
    f8 = ml_dtypes.float8_e4m3

    # x slices: [1024b, 1024i] -> [1024i, 1024b] bf16, partition-major
    xt2s = [
        np.ascontiguousarray(
            x[bg * BS:(bg + 1) * BS, :].T
            .astype(ml_dtypes.bfloat16)
            .reshape(NK, 128, BS)
            .transpose(1, 0, 2)
            .reshape(128, NK * BS)
        )
        for bg in range(BW)
    ]
    c8s, cbs, biascs = [], [], []
    for og in range(OW):
        C = coeffs[og * OS:(og + 1) * OS, :, :]  # [512o, 1024i, 5]
        # fp8 planes p1,p2 scaled: [1024i, 2p, 512o] -> [128,8k,4ot,2p,128o]
        c12 = np.ascontiguousarray(
            (C[:, :, 1:3] * CSC).transpose(1, 2, 0)
        ).astype(f8)  # [1024i, 2, 512o]
        c12 = c12.reshape(NK, 128, 2, NT, 128).transpose(1, 0, 3, 2, 4)
        c8s.append(np.ascontiguousarray(c12))
        # bf16 planes p3,p4: [1024i, 2p, 512o] -> [128, 8k*2p*512o]
        c34 = (
            C[:, :, 3:5].transpose(1, 2, 0)
            .astype(ml_dtypes.bfloat16)
            .reshape(NK, 128, 2 * OS)
            .transpose(1, 0, 2)
            .reshape(128, NK * 2 * OS)
        )
        cbs.append(np.ascontiguousarray(c34))
        bc = (
            bias[0, og * OS:(og + 1) * OS] + C[:, :, 0].sum(axis=1)
        ).astype(np.float32)
        biascs.append(np.ascontiguousarray(bc.reshape(NT, 128).T))
    in_maps = []
    for c in range(BW * OW):
        bg, og = c // OW, c % OW
        in_maps.append(
            {"xt2": xt2s[bg], "c8": c8s[og], "cb": cbs[og],
             "biasc": biascs[og]}
        )
    return in_maps


def _gather(results):
    y = np.empty((B, O), dtype=np.float32)
    for c, res in enumerate(results):
        bg, og = c // OW, c % OW
        y[bg * BS:(bg + 1) * BS, og * OS:(og + 1) * OS] = res["yt"].T
    return y


def run(x, coeffs, bias, trace=False, **trace_kwargs):
    nc = _get_nc()
    in_maps = _make_in_maps(x, coeffs, bias)
    br = run_bass_kernel_spmd(
        nc, in_maps, list(range(BW * OW)), trace=trace, **trace_kwargs
    )
    return _gather(br.results), br


def kernel(x, coeffs, bias):
    out, _ = run(x, coeffs, bias)
    return out


# revision 26
# speedup vs baseline: 1.2274x; 1.1679x over previous
"""Trainium2 Bass kernel for KANPolyLayer:
    y[b,o] = sum_{i,p} x[b,i]^p * coeffs[o,i,p] + bias[o],  p = 0..4

Math: y = sum_{p=1..4} (x^p) @ C_p^T + biascol, with C_p = coeffs[:,:,p]
and biascol[o] = bias[o] + sum_i coeffs[o,i,0] folded on host (the p=0
plane is a constant column; folding it is 0.003% of the FLOPs).

Two-phase mixed-precision stream (the p1/p2 planes carry only ~3% of
the output variance, so fp8 there costs little accuracy but halves
their PE time via DoubleRow):

  Phase A (fp8 DoubleRow): y12 = p1 @ C1'^T + p2 @ C2'^T with
    C' = 256*C cast to fp8e4m3 on host; on-chip p1 = DVE cast of the
    bf16 x plane, p2 = ACT square straight to fp8.  DoubleRow packs
    the (p1,p2) pair as one 256-deep contraction per matmul at 2 fp8
    cols/cycle -> 64 matmuls instead of 128.  Per-bank mid-evacuation
    folds the 1/256 descale and the bias into one ACT:
    stash = psA * (1/256) + biascol.
  Phase B (bf16): psB accumulates p3/p4 (DVE muls); final DVE add of
    psB + stash, in column halves so each half's output DMA overlaps
    the other half's add.

Measured end-to-end rel err ~1.0e-2 (gate 2e-2; verified against an
offline simulation of the same rounding chain).

Schedule notes (from perfetto/NTFF analysis):
- The DMA engines fair-share across all in-flight descriptors, so the
  first tiles' arrival is ruled by how few descriptors are in flight:
  each queue opens with a tiny critical descriptor and later bulk
  loads are issued behind data-gated compute (scalar squares wait on
  x, so descriptors queued after them cannot flood the early window).
- One resident x tile serves both phases; its regions stream in
  behind the gated squares, and the phase-B coeff block loads mostly
  during phase B, whose DMA window is otherwise idle (phase A is
  delivery-rate-bound: ~3MB must land inside its ~17us window).
- Engine balance: phase A: scalar = squares-to-fp8, vector = p1 fp8
  casts + x^2 bf16 muls (phase-B feed).  Phase B: vector = p3/p4 muls
  then the final adds.  Mid-evacs on scalar.
- PE warmup matmuls read broadcast const-APs (written by the framework
  preamble) so they start the moment the PE exits the preamble and
  the HAM clock-gate reaches 2.4 GHz as the real stream begins.
- 8 PSUM banks (4 o-tiles x 2 b-halves); each phase's last NTAIL
  k-planes are emitted bank-contiguous so banks finish staggered and
  evacuation overlaps the stream.

The kernel computes yT = [o, b]; host transposes.

Sharding (8 cores): 4 batch groups x 2 out-dim groups.
  core c -> (bg, og) = (c // 2, c % 2)
Each core computes a disjoint (512 x 1024) block of yT; host gathers.
"""

from contextlib import ExitStack

import ml_dtypes
import numpy as np

import concourse.bacc as bacc
import concourse.bass as bass
import concourse.mybir as mybir
import concourse.tile as tile
from concourse.bass_utils import run_bass_kernel_spmd

F32 = mybir.dt.float32
BF16 = mybir.dt.bfloat16
FP8 = mybir.dt.float8e4

B, I, O = 4096, 1024, 1024  # batch, in_dim, out_dim
BW, OW = 4, 2               # batch groups x out-dim groups (8 cores)
BS, OS = B // BW, O // OW   # per-core batch (1024) and out (512)
NK = I // 128               # contraction tiles (8)
NT = OS // 128              # o-tiles (4)
NH = BS // 512              # b-halves (2)
NTAIL = 2                   # trailing k-planes emitted bank-contiguous
WN = 13                     # PE warmup matmuls (HAM clock-gate)
CSC = 256.0                 # fp8 coeff scale (power of 2, exact)

_CACHE: dict = {}


def _build():
    nc = bacc.Bacc("TRN2", target_bir_lowering=False, debug=False, num_devices=8)

    # x partition-major [i, k*b]: one resident tile serves both phases
    xt2 = nc.dram_tensor("xt2", [128, NK * BS], BF16, kind="ExternalInput")
    # fp8 coeffs for phase A: [i, k, ot, p12, o] = 256*C{1,2}
    c8 = nc.dram_tensor("c8", [128, NK, NT, 2, 128], FP8, kind="ExternalInput")
    # bf16 coeffs for phase B: [i, k*p34*o] partition-major
    cb = nc.dram_tensor("cb", [128, NK * 2 * OS], BF16, kind="ExternalInput")
    # [i, ot]: bias[o] + colsum(C0)[o] as per-partition scalars
    biasc = nc.dram_tensor("biasc", [128, NT], F32, kind="ExternalInput")
    yt = nc.dram_tensor("yt", [OS, BS], F32, kind="ExternalOutput")  # [o, b]

    with tile.TileContext(nc) as tc, ExitStack() as ctx:
        cons = ctx.enter_context(tc.tile_pool(name="cons", bufs=1))
        c8pool = ctx.enter_context(tc.tile_pool(name="c8p", bufs=1))
        cbpool = ctx.enter_context(tc.tile_pool(name="cbp", bufs=1))
        x2pool = ctx.enter_context(tc.tile_pool(name="x2in", bufs=1))
        ppool = ctx.enter_context(tc.tile_pool(name="pow", bufs=1))
        spool = ctx.enter_context(tc.tile_pool(name="stash", bufs=1))
        opool = ctx.enter_context(tc.tile_pool(name="out", bufs=3))
        pspool = ctx.enter_context(
            tc.tile_pool(name="ps", bufs=8, space=bass.MemorySpace.PSUM)
        )

        # 8 accumulation banks: (o-tile, b-half)
        ps = {}
        for ot in range(NT):
            for h in range(NH):
                ps[(ot, h)] = pspool.tile(
                    [128, 512], F32, tag="ps", name=f"ps_{ot}_{h}"
                )

        # PE warmup on framework-preamble const tiles (no memset dep)
        wl = nc.const_aps.tensor(1.0, [128, 128], BF16)
        wr = nc.const_aps.tensor(1.0, [128, 512], BF16)
        for w in range(WN):
            nc.tensor.matmul(
                ps[(0, 0)], wl, wr, start=True, stop=True,
                skip_group_check=True,
            )

        biasc_sb = cons.tile([128, NT], F32)

        # --- DMA: tiny critical descriptors first, bulk gated behind
        # data-dependent compute (scalar squares wait on x, so descriptors
        # queued after them cannot flood the early window) ---
        c8t = c8pool.tile([128, NK, NT, 2, 128], FP8, tag="c8t", name="c8t")
        cbt = cbpool.tile([128, NK * 2 * OS], BF16, tag="cbt", name="cbt")
        x2t = x2pool.tile([128, NK * BS], BF16, tag="x2t", name="x2t")

        # both queues open with an x plane (first data = most critical);
        # fp8 coeffs staged fine -> coarse behind them on the ACT queue
        nc.sync.dma_start(x2t[:, 0:BS], xt2[:, 0:BS])
        nc.scalar.dma_start(x2t[:, BS:2 * BS], xt2[:, BS:2 * BS])
        nc.scalar.dma_start(c8t[:, 0, 0], c8[:, 0, 0])          # k0/ot0, 32KB
        nc.scalar.dma_start(c8t[:, 0, 1:NT], c8[:, 0, 1:NT])    # k0 rest
        nc.scalar.dma_start(c8t[:, 1], c8[:, 1])                # k1
        nc.scalar.dma_start(c8t[:, 2:4], c8[:, 2:4])            # k2-3

        # ---------------- phase A: fp8 DoubleRow (p1, p2) ----------------
        pw12 = {}   # (k, h) -> [128, 2, 512] fp8
        p2bs = {}   # (k, h) -> [128, 512] bf16
        for k in range(NK):
            for h in range(NH):
                sl = x2t[:, k * BS + h * 512:k * BS + (h + 1) * 512]
                pw = ppool.tile([128, 2, 512], FP8, tag=f"pw_{k}_{h}",
                                name=f"pw_{k}_{h}")
                nc.vector.tensor_copy(pw[:, 0], sl)   # fp8 cast of x
                nc.scalar.square(pw[:, 1], sl)        # x^2 -> fp8 direct
                p2b = ppool.tile([128, 512], BF16, tag=f"p2_{k}_{h}",
                                 name=f"p2_{k}_{h}")
                nc.vector.tensor_mul(p2b[:], sl, sl)  # x^2 bf16 for phase B
                pw12[(k, h)] = pw
                p2bs[(k, h)] = p2b

            # staged loads behind the k-gated squares (in-order ACT queue)
            # so they never starve the phase-A-critical stream; phase-B
            # coeffs mostly load during phase B (its DMA window is idle)
            if k == 0:
                nc.scalar.dma_start(x2t[:, 2 * BS:3 * BS],
                                    xt2[:, 2 * BS:3 * BS])
            if k == 1:
                nc.scalar.dma_start(x2t[:, 3 * BS:4 * BS],
                                    xt2[:, 3 * BS:4 * BS])
            if k == 2:
                nc.scalar.dma_start(c8t[:, 4:NK], c8[:, 4:NK])      # 512KB
                nc.scalar.dma_start(x2t[:, 4 * BS:6 * BS],
                                    xt2[:, 4 * BS:6 * BS])
            if k == 3:
                nc.scalar.dma_start(x2t[:, 6 * BS:NK * BS],
                                    xt2[:, 6 * BS:NK * BS])
            if k == 4:
                nc.scalar.dma_start(biasc_sb[:], biasc[:])
            if k == 5:
                nc.scalar.dma_start(cbt[:, 0:2 * 2 * OS], cb[:, 0:2 * 2 * OS])

            if k < NK - NTAIL:
                for ot in range(NT):
                    for h in range(NH):
                        nc.tensor.matmul(
                            ps[(ot, h)],
                            c8t[:, k, ot],
                            pw12[(k, h)][:],
                            start=(k == 0),
                            stop=False,
                            perf_mode=mybir.MatmulPerfMode.DoubleRow,
                        )

        # phase-A tail, bank-contiguous + mid-evac (descale + bias, one ACT)
        stash = {}
        for ot in range(NT):
            for h in range(NH):
                for k in range(NK - NTAIL, NK):
                    nc.tensor.matmul(
                        ps[(ot, h)],
                        c8t[:, k, ot],
                        pw12[(k, h)][:],
                        start=False,
                        stop=(k == NK - 1),
                        perf_mode=mybir.MatmulPerfMode.DoubleRow,
                    )
                st = spool.tile([128, 512], F32, tag=f"st_{ot}_{h}",
                                name=f"st_{ot}_{h}")
                nc.scalar.activation(
                    st[:], ps[(ot, h)][:],
                    mybir.ActivationFunctionType.Identity,
                    bias=biasc_sb[:, ot:ot + 1],
                    scale=1.0 / CSC,
                )
                stash[(ot, h)] = st

        # rest of the phase-B coeffs: B's DMA window is otherwise idle
        nc.scalar.dma_start(cbt[:, 2 * 2 * OS:5 * 2 * OS],
                            cb[:, 2 * 2 * OS:5 * 2 * OS])
        nc.scalar.dma_start(cbt[:, 5 * 2 * OS:NK * 2 * OS],
                            cb[:, 5 * 2 * OS:NK * 2 * OS])

        # ---------------- phase B: bf16 (p3, p4) ----------------
        pows = {}
        for k in range(NK):
            for h in range(NH):
                sl2 = x2t[:, k * BS + h * 512:k * BS + (h + 1) * 512]
                p2b = p2bs[(k, h)]
                p3 = ppool.tile([128, 512], BF16, tag=f"p3_{k}_{h}",
                                name=f"p3_{k}_{h}")
                p4 = ppool.tile([128, 512], BF16, tag=f"p4_{k}_{h}",
                                name=f"p4_{k}_{h}")
                nc.vector.tensor_mul(p3[:], p2b[:], sl2)
                nc.vector.tensor_mul(p4[:], p2b[:], p2b[:])
                pows[(3, k, h)] = p3
                pows[(4, k, h)] = p4

            if k < NK - NTAIL:
                for p in (3, 4):
                    for ot in range(NT):
                        for h in range(NH):
                            nc.tensor.matmul(
                                ps[(ot, h)],
                                cbt[:, k * 2 * OS + (p - 3) * OS + ot * 128:
                                    k * 2 * OS + (p - 3) * OS + (ot + 1) * 128],
                                pows[(p, k, h)][:],
                                start=(k == 0 and p == 3),
                                stop=False,
                            )

        # phase-B tail, bank-contiguous + final add + output DMA
        ngroups = NT * NH
        gi = 0
        for ot in range(NT):
            for h in range(NH):
                for k in range(NK - NTAIL, NK):
                    for p in (3, 4):
                        nc.tensor.matmul(
                            ps[(ot, h)],
                            cbt[:, k * 2 * OS + (p - 3) * OS + ot * 128:
                                k * 2 * OS + (p - 3) * OS + (ot + 1) * 128],
                            pows[(p, k, h)][:],
                            start=False,
                            stop=(k == NK - 1 and p == 4),
                        )
                # final add in halves so each half's DMA overlaps the other
                o_sb = opool.tile([128, 512], F32, tag="o_sb", name=f"o_{ot}_{h}")
                nc.vector.tensor_add(
                    o_sb[:, 0:256], ps[(ot, h)][:, 0:256],
                    stash[(ot, h)][:, 0:256]
                )
                nc.sync.dma_start(
                    yt[ot * 128:(ot + 1) * 128, h * 512:h * 512 + 256],
                    o_sb[:, 0:256],
                )
                nc.vector.tensor_add(
                    o_sb[:, 256:512], ps[(ot, h)][:, 256:512],
                    stash[(ot, h)][:, 256:512]
                )
                nc.scalar.dma_start(
                    yt[ot * 128:(ot + 1) * 128, h * 512 + 256:(h + 1) * 512],
                    o_sb[:, 256:512],
                )
                gi += 1

    nc.compile()
    return nc


def _get_nc():
    if "nc" not in _CACHE:
        _CACHE["nc"] = _build()
    return _CACHE["nc"]


def _make_in_maps(x, coeffs, bias):
    x = np.asarray(x, dtype=np.float32)
    coeffs = np.asarray(coeffs, dtype=np.float32)
    bias = np.asarray(bias, dtype=np.float32)
    f8 = ml_dtypes.float8_e4m3

    # x slices: [1024b, 1024i] -> [1024i, 1024b] bf16, partition-major
    xt2s = [
        np.ascontiguousarray(
            x[bg * BS:(bg + 1) * BS, :].T
            .astype(ml_dtypes.bfloat16)
            .reshape(NK, 128, BS)
            .transpose(1, 0, 2)
            .reshape(128, NK * BS)
        )
        for bg in range(BW)
    ]
    c8s, cbs, biascs = [], [], []
    for og in range(OW):
        C = coeffs[og * OS:(og + 1) * OS, :, :]  # [512o, 1024i, 5]
        # fp8 planes p1,p2 scaled: [1024i, 2p, 512o] -> [128,8k,4ot,2p,128o]
        c12 = np.ascontiguousarray(
            (C[:, :, 1:3] * CSC).transpose(1, 2, 0)
        ).astype(f8)  # [1024i, 2, 512o]
        c12 = c12.reshape(NK, 128, 2, NT, 128).transpose(1, 0, 3, 2, 4)
        c8s.append(np.ascontiguousarray(c12))
        # bf16 planes p3,p4: [1024i, 2p, 512o] -> [128, 8k*2p*512o]
        c34 = (
            C[:, :, 3:5].transpose(1, 2, 0)
            .astype(ml_dtypes.bfloat16)
            .reshape(NK, 128, 2 * OS)
            .transpose(1, 0, 2)
            .reshape(128, NK * 2 * OS)
        )
        cbs.append(np.ascontiguousarray(c34))
        bc = (
            bias[0, og * OS:(og + 1) * OS] + C[:, :, 0].sum(axis=1)
        ).astype(np.float32)
        biascs.append(np.ascontiguousarray(bc.reshape(NT, 128).T))
    in_maps = []
    for c in range(BW * OW):
        bg, og = c // OW, c % OW
        in_maps.append(
            {"xt2": xt2s[bg], "c8": c8s[og], "cb": cbs[og],
             "biasc": biascs[og]}
        )
    return in_maps


def _gather(results):
    y = np.empty((B, O), dtype=np.float32)
    for c, res in enumerate(results):
        bg, og = c // OW, c % OW
        y[bg * BS:(bg + 1) * BS, og * OS:(og + 1) * OS] = res["yt"].T
    return y


def run(x, coeffs, bias, trace=False, **trace_kwargs):
    nc = _get_nc()
    in_maps = _make_in_maps(x, coeffs, bias)
    br = run_bass_kernel_spmd(
        nc, in_maps, list(range(BW * OW)), trace=trace, **trace_kwargs
    )
    return _gather(br.results), br


def kernel(x, coeffs, bias):
    out, _ = run(x, coeffs, bias)
    return out
